# revision 40
# baseline (speedup 1.0000x reference)
"""Trainium2 Bass kernel for nn_CNNEncoder (gather -> lin1 -> conv1d -> maxpool -> MLP).

Strategy
--------
Data-parallel over the 1024 = 64*16 sentences: 128 sentences per NeuronCore.

Host-side algebra: the conv (VALID, taps k=0..4) consumes lin1's output, so
lin1 can be folded into the conv weights:
    y[n,:,t] = sum_k (e[n,t+k] @ W1 + b1) @ Wk + conv_b
             = sum_k e[n,t+k] @ (W1 @ Wk)  +  (b1 @ sum_k Wk + conv_b)
with Wk[i,o] = conv_w[o,i,k].  The constant bias `beff` commutes with the
max-over-time, so it is folded into the MLP bias: b2eff = b2 + beff @ W2[:D].

Per core:
  1. indirect-DMA gather of embedding rows (padded to 320 f32 so each row is a
     256B-multiple) -> SBUF [pos=128, sent, 320]
  2. PE transposes -> e_T [ch_chunk, sent, pos]  (channel-major)
  3. conv as 15 PSUM-accumulated matmuls per (o_chunk, 4-sentence block):
     lhsT = Weff_k[i_chunk, o_chunk], rhs = shifted window of e_T, N = 4*124
  4. DVE max over time -> cnn_T [o_chunk, sent]
  5. tail MLP entirely in [ch, sent] layout (tanh via ACT with per-partition
     bias); output written transposed, un-transposed on host.
"""

import sys

sys.path.insert(0, "/opt/trn_rl_repo")

import os
from contextlib import ExitStack

import numpy as np

import concourse.bass as bass
import concourse.mybir as mybir
import concourse.tile as tile
from concourse import bacc, bass_utils

F32 = mybir.dt.float32
F32R = mybir.dt.float32r
BF16 = mybir.dt.bfloat16
I32 = mybir.dt.int32

VOCAB = 100000
D = 300
K = 5
L = 128          # tokens per sentence
NSENT = 1024     # total sentences
NCORES = 8
NS = NSENT // NCORES   # sentences per core = 128
SB = 4                 # sentences per block
NB = NS // SB          # 32 blocks
TP = L - K + 1         # 124 valid conv positions
DPAD = 320             # embedding row padded to 320 f32 = 1280B (256B multiple)
CH = [(0, 128), (128, 256), (256, 300)]  # chunking of the 300-dim channel axes

# 'f32' (exact, 4 cyc/row), 'f32r' (full rate at N>=256), 'bf16'
CONV_DTYPE = os.environ.get("BASS_CONV_DTYPE", "f32r")

_PROGRAM_CACHE = {}


def _build_program(conv_dtype: str) -> bass.Bass:
    nc = bacc.Bacc(None, target_bir_lowering=False)

    w_dt = {"bf16": BF16, "f32r": F32R, "f32": F32}[conv_dtype]
    # dtype of the gather->transpose path; f32r makes PE transposes 1.5 vs 2
    # cycles/row (PE rounds on read, so no extra precision loss vs f32r matmul)
    tr_dt = F32R if conv_dtype == "f32r" else F32

    # ---- per-core DRAM I/O ----
    tid_t = nc.dram_tensor("tid", [L, NS], I32, kind="ExternalInput")       # [pos, sent]
    embp = nc.dram_tensor("embp", [VOCAB, DPAD], tr_dt, kind="ExternalInput")
    weff = nc.dram_tensor("weff", [K, D, D], w_dt, kind="ExternalInput")    # [k, i, o]
    # packed residual conv weights for channels 256:300 (see conv loop):
    # wr01 rows {0:44 -> tap0, 64:108 -> tap1}, wr23 likewise taps 2/3,
    # wr4 rows 0:44 -> tap 4.  Zero rows elsewhere.
    wr01 = nc.dram_tensor("wr01", [128, D], w_dt, kind="ExternalInput")
    wr23 = nc.dram_tensor("wr23", [128, D], w_dt, kind="ExternalInput")
    wr4 = nc.dram_tensor("wr4", [64, D], w_dt, kind="ExternalInput")
    idn = nc.dram_tensor("idn", [L, L], tr_dt, kind="ExternalInput")        # identity
    idsh = nc.dram_tensor("idsh", [L, L], tr_dt, kind="ExternalInput")      # shift-1 identity
    # tail weights with biases folded in as an extra contraction row:
    # w2cat = [W2 (600 rows); b2eff] -> [601, D], w3cat = [W3; b3] -> [301, D]
    w2cat = nc.dram_tensor("w2cat", [2 * D + 1, D], F32R, kind="ExternalInput")
    w3cat = nc.dram_tensor("w3cat", [D + 1, D], F32R, kind="ExternalInput")
    # mention_rep transposed, with a trailing all-ones row (drives the bias rows)
    m_t = nc.dram_tensor("mt", [D + 1, NS], F32R, kind="ExternalInput")     # [ch, sent]
    out_d = nc.dram_tensor("out", [NS, D], F32, kind="ExternalOutput")      # [sent, ch]

    with tile.TileContext(nc) as tc, ExitStack() as ctx:
        const = ctx.enter_context(tc.tile_pool(name="const", bufs=1))
        epool = ctx.enter_context(tc.tile_pool(name="e", bufs=12))
        etpool = ctx.enter_context(tc.tile_pool(name="et", bufs=6))
        pspool = ctx.enter_context(tc.tile_pool(name="ps", bufs=8, space="PSUM"))

        ident = const.tile([128, 128], tr_dt)
        nc.sync.dma_start(out=ident[:], in_=idn[:])
        ident_s1 = const.tile([L, L], tr_dt)
        nc.sync.dma_start(out=ident_s1[:], in_=idsh[:])

        tid_sb = const.tile([L, NS], I32)
        nc.sync.dma_start(out=tid_sb[:], in_=tid_t[:])

        weff_sb = []  # [k][ci] -> [128, D] for the two full 128-channel chunks
        for k in range(K):
            per_c = []
            for c0, c1 in CH[:2]:
                t = const.tile([c1 - c0, D], w_dt, tag=f"weff{k}_{c0}")
                nc.sync.dma_start(out=t[:], in_=weff[k, c0:c1, :])
                per_c.append(t)
            weff_sb.append(per_c)
        wr01_sb = const.tile([128, D], w_dt)
        nc.sync.dma_start(out=wr01_sb[:], in_=wr01[:])
        wr23_sb = const.tile([128, D], w_dt)
        nc.sync.dma_start(out=wr23_sb[:], in_=wr23[:])
        wr4_sb = const.tile([64, D], w_dt)
        nc.sync.dma_start(out=wr4_sb[:], in_=wr4[:])

        # concat_T tiles [i-chunk, sent] for the tail contraction over the
        # 601-row [cnn(300); mention(300); ones] stack.  cnn rows are written
        # by the conv reduce_max; mention/ones rows DMA'd from m_t.
        W2CH = [(0, 128), (128, 256), (256, 384), (384, 512), (512, 601)]
        c_sb = [
            const.tile([c1 - c0, NS], F32R, tag=f"c_{c0}", name=f"c_{c0}")
            for c0, c1 in W2CH
        ]
        nc.sync.dma_start(out=c_sb[2][44:128, :], in_=m_t[0:84, :])
        nc.sync.dma_start(out=c_sb[3][:], in_=m_t[84:212, :])
        nc.sync.dma_start(out=c_sb[4][:], in_=m_t[212:301, :])

        w2cat_sb = []
        for c0, c1 in W2CH:
            t = const.tile([c1 - c0, D], F32R, tag=f"w2c_{c0}", name=f"w2c_{c0}")
            nc.sync.dma_start(out=t[:], in_=w2cat[c0:c1, :])
            w2cat_sb.append(t)

        JCH = [(0, 100), (100, 200), (200, 300)]
        w3cat_sb = []
        for j0, j1 in JCH:
            t = const.tile([j1 - j0, D], F32R, tag=f"w3c_{j0}", name=f"w3c_{j0}")
            nc.sync.dma_start(out=t[:], in_=w3cat[j0:j1, :])
            w3cat_sb.append(t)
        b3row_sb = const.tile([1, D], F32R)
        nc.sync.dma_start(out=b3row_sb[:], in_=w3cat[D : D + 1, :])
        ones_sb = const.tile([1, NS], F32R)
        nc.sync.dma_start(out=ones_sb[:], in_=m_t[D : D + 1, :])

        # ---- main loop over 4-sentence blocks ----
        for b in range(NB):
            # one gather per sentence: idx [128, 1], out [128, DPAD]
            # (multi-index-per-partition indirect DMA is broken on HW)
            e_s = []
            for s in range(SB):
                e_t = epool.tile([L, DPAD], tr_dt, tag="e", name=f"e_{b}_{s}")
                col = b * SB + s
                nc.gpsimd.indirect_dma_start(
                    out=e_t[:],
                    out_offset=None,
                    in_=embp[:],
                    in_offset=bass.IndirectOffsetOnAxis(
                        ap=tid_sb[:, col : col + 1], axis=0
                    ),
                )
                e_s.append(e_t)

            # transpose to channel-major.
            # et[0], et[1]: channels 0:128 / 128:256, [128, SB, 128].
            # et2: rows 0:64 = channels 256:320 (300:320 are zero-padded),
            #      rows 64:128 = same channels POSITION-SHIFTED by +1 (via the
            #      shift-1 identity) so two conv taps can share one matmul.
            et = []
            for ci, (c0, c1) in enumerate(CH[:2]):
                ps_tr = pspool.tile([128, SB, L], tr_dt, tag="ps", name=f"ps_tr{ci}")
                for s in range(SB):
                    nc.tensor.transpose(
                        out=ps_tr[:, s, :],
                        in_=e_s[s][:, c0:c1],
                        identity=ident[:],
                    )
                et_c = etpool.tile([128, SB, L], w_dt, tag="et", name=f"et{ci}")
                nc.scalar.copy(out=et_c[:], in_=ps_tr[:])
                et.append(et_c)
            # band A (unshifted) and band B (pos+1, via shifted identity) both
            # transpose to PSUM base partition 0 (4-byte transposes may not
            # write at a partition offset); band B is then partition-shifted
            # into et2[64:128] by a SBUF->SBUF DMA.
            ps_tr2a = pspool.tile([64, SB, L], tr_dt, tag="ps")
            ps_tr2b = pspool.tile([64, SB, L], tr_dt, tag="ps")
            for s in range(SB):
                nc.tensor.transpose(
                    out=ps_tr2a[:, s, :], in_=e_s[s][:, 256:320], identity=ident[:]
                )
                nc.tensor.transpose(
                    out=ps_tr2b[:, s, :], in_=e_s[s][:, 256:320], identity=ident_s1[:]
                )
            et2 = etpool.tile([128, SB, L], w_dt, tag="et")
            etb = etpool.tile([64, SB, L], w_dt, tag="etb")
            nc.scalar.copy(out=et2[0:64], in_=ps_tr2a[:])
            nc.scalar.copy(out=etb[:], in_=ps_tr2b[:])
            nc.sync.dma_start(out=et2[64:128], in_=etb[:])

            # conv: 13 PSUM-accumulated matmuls per o_chunk, then max over time
            for oi, (o0, o1) in enumerate(CH):
                ps_y = pspool.tile([o1 - o0, SB, TP], F32, tag="ps")
                n = 0

                def mm(lhsT, rhs, idx):
                    nc.tensor.matmul(
                        out=ps_y[:], lhsT=lhsT, rhs=rhs,
                        start=(idx == 0), stop=(idx == 12),
                    )

                for ci in range(2):
                    for k in range(K):
                        mm(weff_sb[k][ci][:, o0:o1], et[ci][:, :, k : k + TP], n)
                        n += 1
                # channels 256:300, taps packed: (0,1), (2,3), (4)
                mm(wr01_sb[:, o0:o1], et2[:, :, 0:TP], n); n += 1
                mm(wr23_sb[:, o0:o1], et2[:, :, 2 : 2 + TP], n); n += 1
                mm(wr4_sb[:, o0:o1], et2[0:64, :, 4 : 4 + TP], n); n += 1
                cnn_rows = c_sb[oi][0 : o1 - o0] if oi == 2 else c_sb[oi][:]
                nc.vector.tensor_reduce(
                    out=cnn_rows[:, b * SB : (b + 1) * SB],
                    in_=ps_y[:],
                    axis=mybir.AxisListType.X,
                    op=mybir.AluOpType.max,
                )

        # ---- tail MLP, f32r full-rate (N=300), biases folded as ones-rows ----
        # h[s, j] = tanh(sum_c concat_T[c, s] * w2cat[c, j])
        ps_h = pspool.tile([NS, D], F32, tag="ps")
        for c, (c0, c1) in enumerate(W2CH):
            nc.tensor.matmul(
                out=ps_h[:],
                lhsT=c_sb[c][:],
                rhs=w2cat_sb[c][:],
                start=(c == 0),
                stop=(c == len(W2CH) - 1),
            )
        h_sb = const.tile([NS, D], F32R)
        nc.scalar.activation(
            out=h_sb[:], in_=ps_h[:], func=mybir.ActivationFunctionType.Tanh
        )

        # transpose h -> h_T [j-chunk, s] for the second contraction
        ht_sb = []
        for jc, (j0, j1) in enumerate(JCH):
            ps_ht = pspool.tile([100, NS], F32R, tag="ps")
            nc.tensor.transpose(out=ps_ht[:], in_=h_sb[:, j0:j1], identity=ident[:])
            ht = const.tile([100, NS], F32R, tag=f"ht_{j0}", name=f"ht_{j0}")
            nc.scalar.copy(out=ht[:], in_=ps_ht[:])
            ht_sb.append(ht)

        # out[s, q] = sum_j h_T[j, s] * w3cat[j, q] + ones[s] * b3[q]
        ps_o = pspool.tile([NS, D], F32, tag="ps")
        for jc in range(3):
            nc.tensor.matmul(
                out=ps_o[:],
                lhsT=ht_sb[jc][:],
                rhs=w3cat_sb[jc][:],
                start=(jc == 0),
                stop=False,
            )
        nc.tensor.matmul(
            out=ps_o[:], lhsT=ones_sb[:], rhs=b3row_sb[:], start=False, stop=True
        )
        out_sb = const.tile([NS, D], F32)
        nc.scalar.copy(out=out_sb[:], in_=ps_o[:])
        nc.sync.dma_start(out=out_d[:], in_=out_sb[:])

    nc.finalize()
    return nc


def get_program(conv_dtype: str = CONV_DTYPE) -> bass.Bass:
    if conv_dtype not in _PROGRAM_CACHE:
        _PROGRAM_CACHE[conv_dtype] = _build_program(conv_dtype)
    return _PROGRAM_CACHE[conv_dtype]


def _prepare_in_maps(inputs: dict) -> list[dict]:
    token_ids = np.asarray(inputs["token_ids"]).astype(np.int32)      # [1024, 128]
    mention = np.asarray(inputs["mention_rep"], dtype=np.float32).reshape(NSENT, D)
    emb = np.asarray(inputs["emb"], dtype=np.float32)
    W1 = np.asarray(inputs["W1"], dtype=np.float64)
    b1 = np.asarray(inputs["b1"], dtype=np.float64)
    conv_w = np.asarray(inputs["conv_w"], dtype=np.float64)           # [o, i, k]
    conv_b = np.asarray(inputs["conv_b"], dtype=np.float64)
    W2 = np.asarray(inputs["W2"], dtype=np.float64)                   # [2D, D]
    b2 = np.asarray(inputs["b2"], dtype=np.float64)
    W3 = np.asarray(inputs["W3"], dtype=np.float32)                   # [j, q]
    b3 = np.asarray(inputs["b3"], dtype=np.float32)

    Wk = conv_w.transpose(1, 0, 2)                                    # [i, o, k]
    weff = np.stack([W1 @ Wk[:, :, k] for k in range(K)])             # [k, i, o]
    beff = b1 @ Wk.sum(axis=2) + conv_b                               # [o]
    b2eff = b2 + beff @ W2[:D]                                        # [j]
    w2cat_h = np.concatenate([W2, b2eff[None, :]], axis=0).astype(np.float32)
    w3cat_h = np.concatenate(
        [W3.astype(np.float64), np.asarray(inputs["b3"], np.float64)[None, :]], axis=0
    ).astype(np.float32)

    wdt = np.float32
    if CONV_DTYPE == "bf16":
        import ml_dtypes

        wdt = ml_dtypes.bfloat16
    weff_h = weff.astype(wdt)

    # packed residual weights (channels 256:300) — layouts match the kernel's
    # et2 tile: rows 0:44 tap A, 64:108 tap B (shift +1 baked into et2 rows 64+)
    wr01_h = np.zeros((128, D), wdt)
    wr01_h[0:44] = weff[0, 256:300].astype(wdt)
    wr01_h[64:108] = weff[1, 256:300].astype(wdt)
    wr23_h = np.zeros((128, D), wdt)
    wr23_h[0:44] = weff[2, 256:300].astype(wdt)
    wr23_h[64:108] = weff[3, 256:300].astype(wdt)
    wr4_h = np.zeros((64, D), wdt)
    wr4_h[0:44] = weff[4, 256:300].astype(wdt)

    # cyclic shift-1 permutation: ones at ((c+1) % L, c) bakes out[:, c] = in[c+1]
    # (column L-1 wraps to position 0 but is never consumed by the conv windows)
    idsh_h = np.zeros((L, L), np.float32)
    idsh_h[(np.arange(L) + 1) % L, np.arange(L)] = 1.0
    idn_h = np.eye(L, dtype=np.float32)

    emb_pad = np.zeros((VOCAB, DPAD), dtype=np.float32)
    emb_pad[:, :D] = emb

    in_maps = []
    for c in range(NCORES):
        sl = slice(c * NS, (c + 1) * NS)
        mt_h = np.ones((D + 1, NS), np.float32)
        mt_h[:D] = mention[sl].T
        in_maps.append(
            {
                "tid": np.ascontiguousarray(token_ids[sl].T),
                "embp": emb_pad,
                "weff": weff_h,
                "wr01": wr01_h,
                "wr23": wr23_h,
                "wr4": wr4_h,
                "idn": idn_h,
                "idsh": idsh_h,
                "w2cat": w2cat_h,
                "w3cat": w3cat_h,
                "mt": mt_h,
            }
        )
    return in_maps


def run(inputs: dict, trace: bool = False, **kwargs):
    """Run the kernel; returns (output [1024, 300] f32, BassKernelResults)."""
    nc = get_program()
    in_maps = _prepare_in_maps(inputs)
    res = bass_utils.run_bass_kernel_spmd(
        nc, in_maps, core_ids=list(range(NCORES)), trace=trace, **kwargs
    )
    out = np.concatenate(
        [np.asarray(r["out"]) for r in res.results], axis=0
    ).astype(np.float32)
    return out, res


def kernel(**inputs) -> np.ndarray:
    out, _ = run(inputs)
    return out
